# revision 43
# baseline (speedup 1.0000x reference)
"""Trainium2 Bass kernel v3 for nn_EntityEncoder (adapters + BiLSTM + proj).

Sharding: 8 cores = 4 batch-quarters x 2 LSTM directions.

v3 key change: the sequential 256-step LSTM (phase 2) is replaced by a
Jacobi fixed-point iteration over the whole sequence:

    sweep m:  gates = z + Whh @ shift(h^{m-1})        (dense matmuls, N=256)
              sf, si, s2g, so = sigmoid(gates)         (tanh via 2*sig(2x)-1)
              u = si * (2*s2g - 1) = si*tanh(g)
              c = scan(c_t = sf_t * c_{t-1} + u_t)     (tensor_tensor_scan,
                                                        exact per channel)
              h^m = so * (2*sigmoid(2c) - 1) = so*tanh(c)

    Convergence factor ~0.2/sweep (measured): 5 sweeps -> 5e-4 residual.
    Sweep 1 has h=0 so it needs no matmuls at all.

Mask handling (as v2): z += NEG*(m-1) saturates all sigmoids to 0 on
masked steps, giving h=c=0 there; valid for monotone masks (fwd: suffix
masked; bwd: prefix masked after the time reversal done in prep).

Gate chunk order on the 2048-gate axis (16 chunks of 128):
  chunks 0-3 = i, 4-7 = g (pre-scaled x2 for the tanh-via-sigmoid trick),
  8-11 = f, 12-15 = o.

Layouts:
  zT   [128, 16, TOK] f16      TOK = item*256 + t  (time fwd/bwd per core)
  Hbuf [128, 2, 4, 8, 257] f16  ping-pong; per item col 0 == 0 (= h_{-1})
  sfb/ub/sob/cb [128, 4, 8, 257] f16 with col 0 == 0 so the per-pair
  scan [P, 2*257] resets state at item boundaries automatically.
"""

import os

import numpy as np

B, S, H, HL, E, L = 32, 256, 1024, 512, 256, 5
G = 4 * HL            # 2048 gate width
NCORES = 8
BC = 8                # batch items per core
TOK = BC * S          # tokens per core
EPS = 1e-5
P = 128
NEG = 30.0            # mask kill bias
NSWEEP = 4            # Jacobi sweeps (sweep 1 is matmul-free)
T1 = S + 1            # 257: per-item column 0 is the zero h_{-1}/c_{-1}

_CACHE = {}
LAST_RUN = {}

# chunk order on the gate axis: i, g, f, o (torch weight order: i f g o)
_GATE_OF_CHUNK = [0] * 4 + [2] * 4 + [1] * 4 + [3] * 4


def _chunk_perm():
    """perm[c*128+p] = original gate index for chunk c, unit p."""
    perm = np.zeros(G, dtype=np.int64)
    for c in range(16):
        gate = _GATE_OF_CHUNK[c]
        sub = c % 4
        u = np.arange(128) + sub * 128
        perm[c * 128:(c + 1) * 128] = gate * HL + u
    return perm


def _build_nc(nsweeps=NSWEEP, phases=(1, 2)):
    import concourse.tile as tile
    import concourse.mybir as mybir
    from concourse import bacc

    dt = mybir.dt
    f32 = dt.float32
    f16 = dt.float16
    f8 = dt.float8e4
    AF = mybir.ActivationFunctionType
    ALU = mybir.AluOpType
    PM = mybir.MatmulPerfMode

    nc = bacc.Bacc(
        "TRN2", target_bir_lowering=False, debug=False, num_devices=NCORES
    )

    # ---------------- I/O ----------------
    xT = nc.dram_tensor("xT", [H, TOK], f16, kind="ExternalInput").ap()
    W1s = nc.dram_tensor("W1s", [BC, H, H], f16, kind="ExternalInput").ap()
    # fused M = Wih @ W2 stationary tiles per item, pre-arranged so each
    # (item, c-quarter) block is one contiguous DMA: [i, q4, p, kd, c4, u]
    Ms = nc.dram_tensor("Ms", [BC, 4, P, 8, 4, P], f16,
                        kind="ExternalInput").ap()
    # rows 0..2 are b1, ln_g, ln_b; col = item*8 + feat_chunk
    bcols_d = nc.dram_tensor(
        "bcols", [4, P, BC * 8], f32, kind="ExternalInput"
    ).ap()
    # z output bias per (feat_in_chunk, item, chunk): Wih@b2 + b
    zbcols_d = nc.dram_tensor(
        "zbcols", [P, BC, 16], f32, kind="ExternalInput"
    ).ap()
    # mask row: negS [1, G] stationary, mrow [1, TOK] = m-1 moving
    negS = nc.dram_tensor("negS", [1, G], f16, kind="ExternalInput").ap()
    mrow = nc.dram_tensor("mrow", [1, TOK], f16, kind="ExternalInput").ap()
    WhhS = nc.dram_tensor("WhhS", [4, 16, P, P], f16, kind="ExternalInput").ap()
    Whh8 = nc.dram_tensor("Whh8", [2, 16, P, 2, P], f8,
                          kind="ExternalInput").ap()
    # Wp stationary: [k, ec, feat128, e128]
    WpS = nc.dram_tensor("WpS", [4, 2, P, P], f16, kind="ExternalInput").ap()
    I128 = nc.dram_tensor("I128", [P, P], f16, kind="ExternalInput").ap()
    OnesP = nc.dram_tensor("OnesP", [P, P], f16, kind="ExternalInput").ap()
    partial = nc.dram_tensor(
        "partial", [2, P, TOK], f16, kind="ExternalOutput"
    ).ap()

    with tile.TileContext(nc) as tc:
        with tc.tile_pool(name="persist", bufs=1) as persist:
            bcols = persist.tile([P, 4, BC * 8], f32)
            nc.sync.dma_start(out=bcols, in_=bcols_d.rearrange("s p c -> p s c"))
            i128_sb = persist.tile([P, P], f16)
            nc.sync.dma_start(out=i128_sb, in_=I128)
            onesp = persist.tile([P, P], f16)
            nc.sync.dma_start(out=onesp, in_=OnesP)
            eps_sb = persist.tile([P, 1], f32)
            nc.vector.memset(eps_sb, EPS)

            # z resident in SBUF: [128, chunk, token] fp16
            zT = persist.tile([P, 16, TOK], f16)

            # ===== PHASE 1: adapters with fused M = Wih @ W2 ==========
            with (
                tc.tile_pool(name="p1w", bufs=4) as p1w,
                tc.tile_pool(name="p1m", bufs=3) as p1m,
                tc.tile_pool(name="p1misc", bufs=1) as p1misc,
                tc.tile_pool(name="p1x", bufs=2) as p1x,
                tc.tile_pool(name="p1a", bufs=2) as p1a,
                tc.tile_pool(name="p1r", bufs=2) as p1r,
                tc.tile_pool(name="psA", bufs=3, space="PSUM") as psA,
                tc.tile_pool(name="psS", bufs=2, space="PSUM") as psS,
                tc.tile_pool(name="psZ", bufs=3, space="PSUM") as psZ,
            ):
                mrow_sb = p1misc.tile([1, TOK], f16)
                nc.sync.dma_start(out=mrow_sb, in_=mrow)
                negS_sb = p1misc.tile([1, G], f16)
                nc.sync.dma_start(out=negS_sb, in_=negS)
                zbcols = p1misc.tile([P, BC, 16], f32)
                nc.sync.dma_start(out=zbcols, in_=zbcols_d)

                def emit_h1(i):
                    xi = p1x.tile([P, 8, S], f16, tag="xi", name=f"xi{i}")
                    nc.sync.dma_start(
                        out=xi,
                        in_=xT[:, i * S:(i + 1) * S].rearrange(
                            "(k p) t -> p k t", p=P
                        ),
                    )
                    a0 = p1a.tile([P, 8, S], f16, tag="a0", name=f"a0_{i}")
                    sps0 = psS.tile([P, S], f32, tag="sps0", bufs=1,
                                    name=f"sps0_{i}")
                    sps1 = psS.tile([P, S], f32, tag="sps1", bufs=1,
                                    name=f"sps1_{i}")
                    for q4 in range(4):
                        wb = p1w.tile([P, 8, 256], f16, tag="w",
                                      name=f"w1b{i}_{q4}")
                        nc.sync.dma_start(
                            out=wb,
                            in_=W1s[i, :, q4 * 256:(q4 + 1) * 256].rearrange(
                                "(k p) m -> p k m", p=P
                            ),
                        )
                        for mm in range(2):
                            m = q4 * 2 + mm
                            ps = psA.tile([P, S], f32, tag="mm",
                                          name=f"ps1_{i}_{m}")
                            for k in range(8):
                                nc.tensor.matmul(
                                    ps, wb[:, k, mm * P:(mm + 1) * P],
                                    xi[:, k, :],
                                    start=(k == 0), stop=(k == 7),
                                )
                            nc.scalar.activation(
                                out=a0[:, m, :], in_=ps, func=AF.Identity,
                                bias=bcols[:, 0, i * 8 + m: i * 8 + m + 1],
                            )
                            sq = p1a.tile([P, S], f16, tag="sq",
                                          name=f"sq{i}_{m}")
                            nc.scalar.activation(
                                out=sq, in_=a0[:, m, :], func=AF.Square,
                            )
                            nc.tensor.matmul(
                                sps0, onesp, a0[:, m, :],
                                start=(m == 0), stop=(m == 7),
                                skip_group_check=True,
                            )
                            nc.tensor.matmul(
                                sps1, onesp, sq,
                                start=(m == 0), stop=(m == 7),
                                skip_group_check=True,
                            )
                    mrB = p1r.tile([P, 2, S], f32, tag="mrB",
                                   name=f"mrB{i}")
                    nc.scalar.activation(
                        out=mrB[:, 0, :], in_=sps0,
                        func=AF.Identity, scale=1.0 / H,
                    )
                    nc.scalar.activation(
                        out=mrB[:, 1, :], in_=sps1,
                        func=AF.Identity, scale=1.0 / H,
                    )
                    scr = p1r.tile([P, S], f32, tag="scr", name=f"scr{i}")
                    nc.vector.tensor_mul(scr, mrB[:, 0, :], mrB[:, 0, :])
                    nc.vector.tensor_sub(scr, mrB[:, 1, :], scr)
                    nc.scalar.activation(out=mrB[:, 1, :], in_=scr,
                                         func=AF.Abs_reciprocal_sqrt,
                                         bias=eps_sb)
                    return a0, mrB

                def emit_rest(i, a0, mrB):
                    """LN + relu, then fused z = M @ a1 + mask + bias."""
                    isl = slice(i * S, (i + 1) * S)
                    a1 = p1a.tile([P, 8, S], f16, tag="a1", name=f"a1_{i}")
                    for m in range(8):
                        nc.vector.tensor_sub(
                            a1[:, m, :], a0[:, m, :], mrB[:, 0, :]
                        )
                        nc.vector.tensor_mul(
                            a1[:, m, :], a1[:, m, :], mrB[:, 1, :]
                        )
                        nc.vector.tensor_scalar(
                            out=a1[:, m, :], in0=a1[:, m, :],
                            scalar1=bcols[:, 1, i * 8 + m: i * 8 + m + 1],
                            scalar2=bcols[:, 2, i * 8 + m: i * 8 + m + 1],
                            op0=ALU.mult, op1=ALU.add,
                        )
                        nc.scalar.activation(
                            out=a1[:, m, :], in_=a1[:, m, :], func=AF.Relu,
                        )

                    for q4 in range(4):
                        mb = p1m.tile([P, 8, 4, P], f16, tag="m",
                                      name=f"mb{i}_{q4}")
                        nc.sync.dma_start(out=mb, in_=Ms[i, q4])
                        for cp in range(2):  # chunk pairs -> one PSUM bank
                            zp = psZ.tile([P, 2, S], f32, tag="zp",
                                          name=f"zp{i}_{q4}_{cp}")
                            for cc in range(2):
                                c = 4 * q4 + 2 * cp + cc
                                nc.tensor.matmul(
                                    zp[:, cc, :],
                                    negS_sb[:, c * P:(c + 1) * P],
                                    mrow_sb[:, isl],
                                    start=(cc == 0), stop=False,
                                    skip_group_check=True,
                                )
                            for cc in range(2):
                                c = 4 * q4 + 2 * cp + cc
                                for k in range(8):
                                    nc.tensor.matmul(
                                        zp[:, cc, :],
                                        mb[:, k, 2 * cp + cc, :],
                                        a1[:, k, :],
                                        start=False,
                                        stop=(k == 7 and cc == 1),
                                        skip_group_check=True,
                                    )
                            for cc in range(2):
                                c = 4 * q4 + 2 * cp + cc
                                nc.scalar.activation(
                                    out=zT[:, c, isl], in_=zp[:, cc, :],
                                    func=AF.Identity,
                                    bias=zbcols[:, i, c:c + 1],
                                )

                if 1 in phases:
                    pending = emit_h1(0)
                    for i in range(BC):
                        nxt = emit_h1(i + 1) if i + 1 < BC else None
                        emit_rest(i, *pending)
                        pending = nxt

            # ================= PHASE 2: Jacobi sweeps =================
            with (
                tc.tile_pool(name="p2whh", bufs=1) as p2whh,
                tc.tile_pool(name="p2st", bufs=1) as p2st,
                tc.tile_pool(name="p2sig", bufs=1) as p2sig,
                tc.tile_pool(name="p2a", bufs=1) as p2a,
                tc.tile_pool(name="p2sc", bufs=2) as p2sc,
                tc.tile_pool(name="ps2", bufs=2, space="PSUM") as ps2,
            ):
                whh_sb = p2whh.tile([P, 4, 16, P], f16)
                nc.sync.dma_start(
                    out=whh_sb, in_=WhhS.rearrange("k c p u -> p k c u")
                )
                whh8_sb = p2whh.tile([P, 2, 16, 2, P], f8)
                nc.sync.dma_start(
                    out=whh8_sb, in_=Whh8.rearrange("k c p o u -> p k c o u")
                )
                wp_sb = p2whh.tile([P, 4, 2, P], f16)
                nc.sync.dma_start(
                    out=wp_sb, in_=WpS.rearrange("k e p m -> p k e m")
                )

                # H buffers: sweeps 0,1 emit fp8 (consumed by the fp8
                # DoubleRow sweeps 1,2); sweeps 2,3 emit fp16
                Hb = [
                    p2st.tile([P, 4, BC, T1], f8, name="H8a"),
                    p2st.tile([P, 4, BC, T1], f8, name="H8b"),
                    p2st.tile([P, 4, BC, T1], f8, name="H8c"),
                    p2st.tile([P, 4, BC, T1], f16, name="H16b"),
                ]
                sfb = p2st.tile([P, 4, BC, T1], f16)
                ub = p2st.tile([P, 4, BC, T1], f16)
                sob = p2st.tile([P, 4, BC, T1], f16)
                # only col 0 (the zero h_{-1}/c_{-1} slot) must be zeroed;
                # cols 1..256 are rewritten every sweep before being read
                for t_ in (sfb, ub, sob):
                    nc.vector.memset(t_[:, :, :, 0:1], 0.0)

                def emit_item(m, i, Hr, fp8_mm):
                    """gate waves + sigmoids + u for item i, sweep m.

                    z/Whh carry a x32 scale; every gate sigmoid applies
                    scale=1/32.
                    """
                    isl = slice(i * S, (i + 1) * S)
                    for w in range(2):
                        if m == 0:
                            src = zT[:, w * 8:(w + 1) * 8, isl]
                        else:
                            pw = ps2.tile([P, 8, S], f32, tag="pw",
                                          name=f"pw{m}_{i}_{w}")
                            # one accumulation group per 2KB PSUM bank
                            # (chunk pair): start=True on a sub-bank slice
                            # clobbers the whole bank's has_written state
                            for b8 in range(4):
                                c = w * 8 + 2 * b8
                                nc.tensor.matmul(
                                    pw[:, 2 * b8:2 * b8 + 2, :], i128_sb,
                                    zT[:, c:c + 2, isl],
                                    start=True, stop=False,
                                    skip_group_check=True,
                                )
                            for c8 in range(8):
                                c = w * 8 + c8
                                if fp8_mm:
                                    for kc in range(2):
                                        nc.tensor.matmul(
                                            pw[:, c8, :],
                                            whh8_sb[:, kc, c, :, :],
                                            Hr[:, 2 * kc:2 * kc + 2, i, 0:S],
                                            start=False,
                                            stop=(kc == 1 and c8 % 2 == 1),
                                            skip_group_check=True,
                                            perf_mode=PM.DoubleRow,
                                        )
                                else:
                                    for k in range(4):
                                        nc.tensor.matmul(
                                            pw[:, c8, :], whh_sb[:, k, c, :],
                                            Hr[:, k, i, 0:S],
                                            start=False,
                                            stop=(k == 3 and c8 % 2 == 1),
                                            skip_group_check=True,
                                        )
                            src = pw
                        if w == 0:
                            sig8 = p2sig.tile([P, 8, S], f16, tag="sig8",
                                              name=f"sig{m}_{i}")
                            nc.scalar.activation(
                                out=sig8, in_=src, func=AF.Sigmoid,
                                scale=1.0 / 32.0,
                            )
                            a = p2a.tile([P, 4, S], f16, tag="a",
                                         name=f"a{m}_{i}")
                            nc.vector.tensor_mul(
                                a, sig8[:, 0:4, :], sig8[:, 4:8, :]
                            )
                            nc.vector.scalar_tensor_tensor(
                                out=ub[:, :, i, 1:T1], in0=a, scalar=2.0,
                                in1=sig8[:, 0:4, :],
                                op0=ALU.mult, op1=ALU.subtract,
                            )
                        else:
                            nc.scalar.activation(
                                out=sfb[:, :, i, 1:T1],
                                in_=src[:, 0:4, :],
                                func=AF.Sigmoid, scale=1.0 / 32.0,
                            )
                            nc.scalar.activation(
                                out=sob[:, :, i, 1:T1],
                                in_=src[:, 4:8, :],
                                func=AF.Sigmoid, scale=1.0 / 32.0,
                            )

                def emit_pair_tail(m, pr, Hw):
                    """scan + h for items 2pr, 2pr+1."""
                    psl = slice(2 * pr, 2 * pr + 2)
                    cbp = p2sc.tile([P, 4, 2, T1], f16, tag="cbp",
                                    name=f"cb{m}_{pr}")
                    for k in range(4):
                        nc.vector.tensor_tensor_scan(
                            out=cbp[:, k].rearrange("p i t -> p (i t)"),
                            data0=sfb[:, k, psl, :].rearrange(
                                "p i t -> p (i t)"),
                            data1=ub[:, k, psl, :].rearrange(
                                "p i t -> p (i t)"),
                            initial=0.0,
                            op0=ALU.mult, op1=ALU.add,
                        )
                    sc = p2sc.tile([P, 4, 2, T1], f16, tag="sc", bufs=1,
                                   name=f"sc{m}_{pr}")
                    nc.scalar.activation(
                        out=sc.rearrange("p k i t -> p (k i t)"),
                        in_=cbp.rearrange("p k i t -> p (k i t)"),
                        func=AF.Sigmoid, scale=2.0,
                    )  # both tiles contiguous -> mergeable
                    r = p2sc.tile([P, 4, 2, T1], f16, tag="r", bufs=1,
                                  name=f"r{m}_{pr}")
                    nc.vector.tensor_mul(r, sob[:, :, psl, :], sc)
                    nc.vector.scalar_tensor_tensor(
                        out=Hw[:, :, psl, :], in0=r, scalar=2.0,
                        in1=sob[:, :, psl, :],
                        op0=ALU.mult, op1=ALU.subtract,
                    )

                if 2 in phases:
                    assert nsweeps == 4
                    for m in range(nsweeps):
                        Hr = Hb[m - 1] if m > 0 else None
                        Hw = Hb[m]
                        fp8_mm = m >= 1
                        # tails deferred one pair so sigma(2c) doesn't sit
                        # ahead of the next pair's gate sigmoids in the
                        # in-order scalar queue (stalls the PSUM recycle)
                        for pr in range(4):
                            emit_item(m, 2 * pr, Hr, fp8_mm)
                            emit_item(m, 2 * pr + 1, Hr, fp8_mm)
                            if pr > 0:
                                emit_pair_tail(m, pr - 1, Hw)
                        emit_pair_tail(m, 3, Hw)

                    # ---------- projection out = Wp.T @ h ----------
                    # psum reuses the ps2 "pw" ring ([P,8,256] = 4 banks;
                    # only the first 512 fp32 are used per tile)
                    Hf = Hb[nsweeps - 1]
                    for ec in range(2):
                        for tck in range(4):
                            ppt = ps2.tile([P, 8, S], f32, tag="pw",
                                           name=f"pp{ec}_{tck}")
                            pp = ppt[:, 0:2, :].rearrange(
                                "p c t -> p (c t)")
                            mv = Hf[:, :, 2 * tck:2 * tck + 2, 1:T1]
                            for k in range(4):
                                nc.tensor.matmul(
                                    pp, wp_sb[:, k, ec, :],
                                    mv[:, k],
                                    start=(k == 0), stop=(k == 3),
                                )
                            obt = p2sig.tile([P, 8, S], f16, tag="sig8",
                                             name=f"ob{ec}_{tck}")
                            ob = obt[:, 0:2, :].rearrange("p c t -> p (c t)")
                            nc.scalar.activation(
                                out=ob, in_=pp, func=AF.Identity,
                            )
                            nc.sync.dma_start(
                                out=partial[ec, :,
                                            tck * 512:(tck + 1) * 512],
                                in_=ob,
                            )

    nc.finalize()
    return nc


def _prep_core_inputs(core, perm, seq, am, li, W1, b1, ln_g, ln_b,
                      Mlang, zblang, Whh, Wp):
    """Mlang[l] = Wih_dir @ W2[l].T [G, H]; zblang[l] = Wih_dir@b2[l]+b."""
    q = core % 4
    bwd = core >= 4
    items = perm[q * BC:(q + 1) * BC]
    cperm = _chunk_perm()
    # x2 scale on g-chunk rows (chunks 4..7) for tanh-via-sigmoid
    gscale = np.ones(G, dtype=np.float32)
    gscale[4 * P:8 * P] = 2.0

    x = seq[items]                          # [8, S, H]
    mm = am[items].astype(np.float32)       # [8, S]
    if bwd:
        x = x[:, ::-1, :]
        mm = mm[:, ::-1]
    xT = np.ascontiguousarray(
        x.transpose(2, 0, 1).reshape(H, TOK), dtype=np.float16
    )
    langs = li[items]
    W1s = np.ascontiguousarray(W1[langs], dtype=np.float16)

    def cols(v):                            # [L,1024] -> [128, item*8+m]
        vv = v[langs]
        return vv.reshape(BC, 8, P).transpose(2, 0, 1).reshape(P, BC * 8)

    bcols = np.ascontiguousarray(
        np.stack([cols(b1), cols(ln_g), cols(ln_b), cols(ln_b)], axis=0),
        dtype=np.float32,
    )

    import ml_dtypes

    # fused M stationary tiles per item [kd, c, feat128, unit128];
    # g-rows x2, all x32 (undone by scale=1/32 in the gate sigmoids)
    sc = 32.0 * gscale
    Ms = np.empty((BC, 4, P, 8, 4, P), dtype=np.float16)
    zbc = np.empty((P, BC, 16), dtype=np.float32)
    for j, l in enumerate(langs):
        Mp = Mlang[l][cperm, :] * sc[:, None]
        tiles = Mp.reshape(16, P, 8, P).transpose(2, 0, 3, 1)  # [k,c,f,u]
        Ms[j] = tiles.reshape(8, 4, 4, P, P).transpose(
            1, 3, 0, 2, 4)  # [q4, p, k, c4, u]
        zbc[:, j, :] = (zblang[l][cperm] * sc).reshape(16, P).T
    negS = (NEG * 32.0 * gscale)[None, :].astype(np.float16)
    mrow = (mm - 1.0).reshape(1, TOK).astype(np.float16)

    WhhP = Whh[cperm, :] * (32.0 * gscale[:, None])
    WhhS32 = WhhP.reshape(16, P, 4, P).transpose(2, 0, 3, 1)  # [k,c,f,u]
    WhhS = np.ascontiguousarray(WhhS32, dtype=np.float16)
    # DoubleRow fp8 tiles [kc, c, ki, ko, m]: feat = ko*128 + ki
    Whh8 = np.ascontiguousarray(
        WhhS32.reshape(2, 2, 16, P, P).transpose(0, 2, 3, 1, 4),
        dtype=np.float32,
    ).astype(ml_dtypes.float8_e4m3fn)

    d0 = HL if bwd else 0
    WpS = np.ascontiguousarray(
        Wp[:, d0:d0 + HL].T.reshape(4, P, 2, P).transpose(0, 2, 1, 3),
        dtype=np.float16,
    )  # [k, ec, feat, e]

    return {
        "xT": xT, "W1s": W1s, "Ms": Ms, "bcols": bcols,
        "zbcols": zbc, "negS": negS, "mrow": mrow, "WhhS": WhhS,
        "Whh8": Whh8, "WpS": WpS, "I128": np.eye(P, dtype=np.float16),
        "OnesP": np.ones((P, P), dtype=np.float16),
    }


def kernel(sequence_output, attention_mask, language_ids, W1, b1, ln_g, ln_b,
           W2, b2, Wih_f, Whh_f, b_f, Wih_b, Whh_b, b_b, Wp, bp):
    from concourse.bass_utils import run_bass_kernel_spmd

    seq = np.asarray(sequence_output, dtype=np.float32)
    am = np.asarray(attention_mask)
    li = np.asarray(language_ids).astype(np.int64)

    key = "nc3"
    if key not in _CACHE:
        _CACHE[key] = _build_nc()
    nc = _CACHE[key]

    perm = np.argsort(li, kind="stable")
    # fused adapter-out -> gate projections, shared across cores
    W2_32 = np.asarray(W2, np.float32)
    b2_32 = np.asarray(b2, np.float32)
    Mcache = {}
    for d_, (Wih_d, b_d) in enumerate(
            [(Wih_f, b_f), (Wih_b, b_b)]):
        Wih32 = np.asarray(Wih_d, np.float32)
        b32 = np.asarray(b_d, np.float32)
        Mcache[d_] = (
            {l: Wih32 @ W2_32[l].T for l in range(L)},
            {l: Wih32 @ b2_32[l] + b32 for l in range(L)},
        )
    in_maps = []
    for core in range(NCORES):
        bwd = core >= 4
        Mlang, zblang = Mcache[1 if bwd else 0]
        in_maps.append(
            _prep_core_inputs(
                core, perm, seq, am, li,
                np.asarray(W1, np.float32), np.asarray(b1, np.float32),
                np.asarray(ln_g, np.float32), np.asarray(ln_b, np.float32),
                Mlang, zblang,
                np.asarray(Whh_b if bwd else Whh_f, np.float32),
                np.asarray(Wp, np.float32),
            )
        )

    trace = bool(os.environ.get("KERNEL_TRACE"))
    res = run_bass_kernel_spmd(
        nc, in_maps, core_ids=list(range(NCORES)), trace=trace
    )
    LAST_RUN["exec_time_ns"] = res.exec_time_ns
    LAST_RUN["profile_json"] = res.profile_json
    # partial: [2, 128, TOK] -> [E=256, item, t] -> [item, t, E]
    outs = [
        r["partial"].reshape(E, BC, S).transpose(1, 2, 0)
        for r in res.results
    ]

    out = np.empty((B, S, E), dtype=np.float32)
    bp32 = np.asarray(bp, dtype=np.float32)
    for q in range(4):
        items = perm[q * BC:(q + 1) * BC]
        pf = outs[q]                        # [8, S, E]
        pb = outs[q + 4][:, ::-1, :]        # un-reverse time
        out[items] = pf + pb + bp32
    return out
